# revision 12
# baseline (speedup 1.0000x reference)
"""MoE (top-2 of 8 experts) Trainium2 kernel — two-launch sparse design.

Problem dims: N=16384 tokens, D=512 model, H=2048 hidden, E=8 experts, K=2.

Launch 1 (router, data-parallel): each of the 8 cores takes N/8 = 2048
tokens and computes softmax + top-2 gate weights [2048, 8] on device
(fp32 matmul, ACT exp, DVE max8/match_replace top-k).

Host dispatch: the gate matrix tells which tokens go to which expert;
the host gathers each expert's token rows (transposed, padded to a
common capacity C) — this is the "all-to-all token dispatch" of the
sharding hint, done by the host since kernel IO is full-tensor anyway.

Launch 2 (expert FFN, expert-parallel): core e holds expert e's W1/b1/
W2/b2 and computes gate * (relu(x_e @ W1 + b1) @ W2 + b2) for its C
gathered tokens. Matmuls run in float32r (full PE rate, ~1.5e-4 rel
err); b1 is applied exactly inside the relu; b2 exactly via an extra
ones-row contraction term so the gate multiplies (y + b2) as in the
reference.

Host combine: out[idx_e] += y_e per expert (indices within an expert
are unique, so fancy-index add is exact fp32, matching the reference's
expert-order accumulation).
"""
from contextlib import ExitStack

import numpy as np

N_TOK, D_MODEL, H_HID, N_EXP, TOPK = 16384, 512, 2048, 8, 2
N_CORES = 8
P = 128
T_CORE = N_TOK // N_CORES          # 2048 tokens per router core
N_TCH = T_CORE // P                # 16 token chunks of 128
N_DCH = D_MODEL // P               # 4
N_HCH = H_HID // P                 # 16

_CACHE = {}


def build_router_kernel(reps: int = 1):
    import concourse.bacc as bacc
    import concourse.tile as tile
    import concourse.mybir as mybir

    f32 = mybir.dt.float32
    nc = bacc.Bacc("TRN2", target_bir_lowering=False, debug=False,
                   num_devices=N_CORES)
    xt_d = nc.dram_tensor("xt", [D_MODEL, T_CORE], f32,
                          kind="ExternalInput").ap()
    wr_d = nc.dram_tensor("wr", [D_MODEL, N_EXP], f32,
                          kind="ExternalInput").ap()
    gates_d = nc.dram_tensor("gates", [T_CORE, N_EXP], f32,
                             kind="ExternalOutput").ap()

    with tile.TileContext(nc) as tc:
        with ExitStack() as ctx:
            def body():
                sb = ctx.enter_context(tc.tile_pool(name="sb", bufs=1))
                evpool = ctx.enter_context(tc.tile_pool(name="ev", bufs=3))
                ps = ctx.enter_context(
                    tc.tile_pool(name="ps", bufs=3, space="PSUM"))

                xt_sb = sb.tile([P, N_DCH * T_CORE], f32, tag="xt")
                nc.sync.dma_start(
                    xt_sb[:].rearrange("p (dc t) -> p dc t", dc=N_DCH),
                    xt_d.rearrange("(dc p) t -> p dc t", p=P))
                wr_sb = sb.tile([P, N_DCH * N_EXP], f32, tag="wr")
                nc.sync.dma_start(
                    wr_sb[:].rearrange("p (dc e) -> p dc e", dc=N_DCH),
                    wr_d.rearrange("(dc p) e -> p dc e", p=P))

                gates_sb = sb.tile([P, N_TCH * N_EXP], f32, tag="gates")
                for tch in range(N_TCH):
                    lg_ps = ps.tile([P, N_EXP], f32, tag="lg", space="PSUM")
                    for dc in range(N_DCH):
                        nc.tensor.matmul(
                            lg_ps[:],
                            xt_sb[:, dc * T_CORE + tch * P:
                                  dc * T_CORE + (tch + 1) * P],
                            wr_sb[:, dc * N_EXP:(dc + 1) * N_EXP],
                            start=(dc == 0), stop=(dc == N_DCH - 1))
                    lg = evpool.tile([P, N_EXP], f32, tag="lg_sb")
                    mx = evpool.tile([P, 1], f32, tag="mx")
                    nc.vector.reduce_max(mx[:], lg_ps[:],
                                         axis=mybir.AxisListType.X)
                    nc.vector.tensor_scalar_sub(lg[:], lg_ps[:], mx[:])
                    pr = evpool.tile([P, N_EXP], f32, tag="pr")
                    nc.scalar.activation(
                        pr[:], lg[:], mybir.ActivationFunctionType.Exp)
                    sm = evpool.tile([P, 1], f32, tag="sm")
                    nc.vector.reduce_sum(sm[:], pr[:],
                                         axis=mybir.AxisListType.X)
                    rc = evpool.tile([P, 1], f32, tag="rc")
                    nc.vector.reciprocal(rc[:], sm[:])
                    nc.vector.tensor_scalar_mul(pr[:], pr[:], rc[:])
                    m8 = evpool.tile([P, 8], f32, tag="m8")
                    nc.vector.max(m8[:], pr[:])
                    nc.vector.memset(m8[:, TOPK:], -1.0)
                    rep = evpool.tile([P, N_EXP], f32, tag="rep")
                    nc.vector.match_replace(rep[:], m8[:], pr[:], 0.0)
                    g = gates_sb[:, tch * N_EXP:(tch + 1) * N_EXP]
                    nc.vector.tensor_sub(g, pr[:], rep[:])

                nc.sync.dma_start(
                    gates_d.rearrange("(tc p) e -> p tc e", p=P),
                    gates_sb[:].rearrange("p (tc e) -> p tc e", tc=N_TCH))

            if reps == 1:
                body()
            else:
                with tc.For_i(0, reps, 1):
                    body()
    nc.compile()
    return nc


def build_ffn_kernel(cap: int, reps: int = 1):
    """Expert-parallel FFN: one expert per core, cap tokens (mult of 512)."""
    import concourse.bacc as bacc
    import concourse.tile as tile
    import concourse.mybir as mybir

    assert cap % 512 == 0
    n_tt = cap // 512
    n_sch = cap // P       # slot chunks of 128

    f32 = mybir.dt.float32
    f32r = mybir.dt.float32r
    nc = bacc.Bacc("TRN2", target_bir_lowering=False, debug=False,
                   num_devices=N_CORES)
    xet_d = nc.dram_tensor("xet", [D_MODEL, cap], f32r,
                           kind="ExternalInput").ap()
    w1_d = nc.dram_tensor("w1", [D_MODEL, H_HID], f32r,
                          kind="ExternalInput").ap()
    b1_d = nc.dram_tensor("b1", [H_HID], f32, kind="ExternalInput").ap()
    w2_d = nc.dram_tensor("w2", [H_HID, D_MODEL], f32r,
                          kind="ExternalInput").ap()
    b2_d = nc.dram_tensor("b2", [D_MODEL], f32r, kind="ExternalInput").ap()
    gv_d = nc.dram_tensor("gv", [cap], f32, kind="ExternalInput").ap()
    y_d = nc.dram_tensor("y", [cap, D_MODEL], f32,
                         kind="ExternalOutput").ap()

    with tile.TileContext(nc) as tc:
        with ExitStack() as ctx:
            def body():
                sb = ctx.enter_context(tc.tile_pool(name="sb", bufs=1))
                hpool = ctx.enter_context(tc.tile_pool(name="hp", bufs=3))
                ypool_sb = ctx.enter_context(tc.tile_pool(name="ysb",
                                                          bufs=3))
                ps = ctx.enter_context(
                    tc.tile_pool(name="ps", bufs=3, space="PSUM"))
                yps = ctx.enter_context(
                    tc.tile_pool(name="yp", bufs=1, space="PSUM"))

                # resident loads, spread across the three DMA engines
                xet_sb = sb.tile([P, N_DCH * cap], f32r, tag="xet")
                nc.sync.dma_start(
                    xet_sb[:].rearrange("p (dc t) -> p dc t", dc=N_DCH),
                    xet_d.rearrange("(dc p) t -> p dc t", p=P))
                w1_sb = sb.tile([P, N_DCH * H_HID], f32r, tag="w1")
                nc.gpsimd.dma_start(
                    w1_sb[:].rearrange("p (dc h) -> p dc h", dc=N_DCH),
                    w1_d.rearrange("(dc p) h -> p dc h", p=P))
                w2_sb = sb.tile([P, N_HCH * D_MODEL], f32r, tag="w2")
                nc.scalar.dma_start(
                    w2_sb[:].rearrange("p (hc d) -> p hc d", hc=N_HCH),
                    w2_d.rearrange("(hc p) d -> p hc d", p=P))
                b1_sb = sb.tile([P, N_HCH], f32, tag="b1")
                nc.sync.dma_start(
                    b1_sb[:], b1_d.rearrange("(hc p) -> p hc", p=P))
                b2_sb = sb.tile([1, D_MODEL], f32r, tag="b2")
                nc.sync.dma_start(b2_sb[:], b2_d[None, :])
                gv_sb = sb.tile([P, n_sch], f32, tag="gv")
                nc.sync.dma_start(
                    gv_sb[:], gv_d.rearrange("(sc p) -> p sc", p=P))
                ones_f = sb.tile([1, P], f32, tag="ones_f")
                nc.vector.memset(ones_f[:], 1.0)
                ones_sb = sb.tile([1, P], f32r, tag="ones")
                nc.vector.tensor_copy(ones_sb[:], ones_f[:])

                import concourse.mybir as mybir
                for tt in range(n_tt):
                    y_ps = [yps.tile([P, D_MODEL], f32, tag=f"y{i}",
                                     name=f"y{i}", space="PSUM")
                            for i in range(4)]
                    for hc in range(N_HCH):
                        h_ps = ps.tile([P, 512], f32, tag="h", space="PSUM")
                        for dc in range(N_DCH):
                            nc.tensor.matmul(
                                h_ps[:],
                                w1_sb[:, dc * H_HID + hc * P:
                                      dc * H_HID + (hc + 1) * P],
                                xet_sb[:, dc * cap + tt * 512:
                                       dc * cap + (tt + 1) * 512],
                                start=(dc == 0), stop=(dc == N_DCH - 1))
                        h_sb = hpool.tile([P, 512], f32r, tag="h_sb")
                        nc.vector.tensor_scalar(
                            h_sb[:], h_ps[:],
                            b1_sb[:, hc:hc + 1], 0.0,
                            op0=mybir.AluOpType.add,
                            op1=mybir.AluOpType.max)
                        for ts in range(4):
                            nc.tensor.matmul(
                                y_ps[ts][:],
                                h_sb[:, ts * P:(ts + 1) * P],
                                w2_sb[:, hc * D_MODEL:(hc + 1) * D_MODEL],
                                start=(hc == 0), stop=False)
                    for ts in range(4):
                        nc.tensor.matmul(
                            y_ps[ts][:], ones_sb[:, :P], b2_sb[:],
                            start=False, stop=True)
                    for ts in range(4):
                        sc = tt * 4 + ts
                        y_sb = ypool_sb.tile([P, D_MODEL], f32, tag="ysb")
                        nc.scalar.activation(
                            y_sb[:], y_ps[ts][:],
                            mybir.ActivationFunctionType.Copy,
                            scale=gv_sb[:, sc:sc + 1])
                        nc.sync.dma_start(
                            y_d[sc * P:(sc + 1) * P, :], y_sb[:])

            if reps == 1:
                body()
            else:
                with tc.For_i(0, reps, 1):
                    body()
    nc.compile()
    return nc


def _router_in_maps(input_batch, Wr):
    return [{
        "xt": np.ascontiguousarray(
            input_batch[c * T_CORE:(c + 1) * T_CORE].T),
        "wr": np.ascontiguousarray(Wr),
    } for c in range(N_CORES)]


def _dispatch(input_batch, gates):
    """Build per-expert gathered inputs. Returns (cap, idx_list, in_maps)."""
    idx_list = [np.nonzero(gates[:, e])[0] for e in range(N_EXP)]
    max_cnt = max(len(ix) for ix in idx_list)
    cap = max(512, ((max_cnt + 511) // 512) * 512)
    in_maps = []
    for e in range(N_EXP):
        ix = idx_list[e]
        xe_t = np.zeros((D_MODEL, cap), np.float32)
        xe_t[:, :len(ix)] = input_batch[ix].T
        gv = np.zeros(cap, np.float32)
        gv[:len(ix)] = gates[ix, e]
        in_maps.append({"xet": xe_t, "gv": gv})
    return cap, idx_list, in_maps


def kernel(input_batch, Wr, W1, b1, W2, b2):
    from concourse import bass_utils

    input_batch = np.ascontiguousarray(input_batch, dtype=np.float32)
    Wr = np.ascontiguousarray(Wr, dtype=np.float32)
    W1 = np.ascontiguousarray(W1, dtype=np.float32)
    b1 = np.ascontiguousarray(b1, dtype=np.float32)
    W2 = np.ascontiguousarray(W2, dtype=np.float32)
    b2 = np.ascontiguousarray(b2, dtype=np.float32)

    # ---- launch 1: router ----
    if "router" not in _CACHE:
        _CACHE["router"] = build_router_kernel()
    res = bass_utils.run_bass_kernel_spmd(
        _CACHE["router"], _router_in_maps(input_batch, Wr),
        core_ids=list(range(N_CORES)))
    gates = np.concatenate([res.results[c]["gates"]
                            for c in range(N_CORES)], axis=0)

    # ---- host dispatch ----
    cap, idx_list, in_maps = _dispatch(input_batch, gates)

    # ---- launch 2: expert FFN ----
    key = ("ffn", cap)
    if key not in _CACHE:
        _CACHE[key] = build_ffn_kernel(cap)
    for e in range(N_EXP):
        in_maps[e]["w1"] = W1[e]
        in_maps[e]["b1"] = b1[e]
        in_maps[e]["w2"] = W2[e]
        in_maps[e]["b2"] = b2[e]
    res = bass_utils.run_bass_kernel_spmd(
        _CACHE[key], in_maps, core_ids=list(range(N_CORES)))

    # ---- host combine ----
    out = np.zeros((N_TOK, D_MODEL), np.float32)
    for e in range(N_EXP):
        ix = idx_list[e]
        out[ix] += res.results[e]["y"][:len(ix)]
    return out, np.float32(0.0)


# revision 16
# speedup vs baseline: 1.9556x; 1.9556x over previous
"""MoE (top-2 of 8 experts) Trainium2 kernel — two-launch sparse design.

Problem dims: N=16384 tokens, D=512 model, H=2048 hidden, E=8 experts, K=2.

Launch 1 (router, data-parallel): each of the 8 cores takes N/8 = 2048
tokens and computes softmax + top-2 gate weights [2048, 8] on device
(fp32 matmul, ACT exp, DVE max8/match_replace top-k).

Host dispatch: the gate matrix tells which tokens go to which expert;
the host gathers each expert's token rows (transposed, padded to a
common capacity C) — this is the "all-to-all token dispatch" of the
sharding hint, done by the host since kernel IO is full-tensor anyway.

Launch 2 (expert FFN, expert-parallel): core e holds expert e's W1/b1/
W2/b2 and computes gate * (relu(x_e @ W1 + b1) @ W2 + b2) for its C
gathered tokens. Matmuls run in float32r (full PE rate, ~1.5e-4 rel
err); b1 is applied exactly inside the relu; b2 exactly via an extra
ones-row contraction term so the gate multiplies (y + b2) as in the
reference.

Host combine: out[idx_e] += y_e per expert (indices within an expert
are unique, so fancy-index add is exact fp32, matching the reference's
expert-order accumulation).
"""
from contextlib import ExitStack

import numpy as np

N_TOK, D_MODEL, H_HID, N_EXP, TOPK = 16384, 512, 2048, 8, 2
N_CORES = 8
P = 128
T_CORE = N_TOK // N_CORES          # 2048 tokens per router core
N_TCH = T_CORE // P                # 16 token chunks of 128
N_DCH = D_MODEL // P               # 4
N_HCH = H_HID // P                 # 16

_CACHE = {}


def build_router_kernel(reps: int = 1):
    import concourse.bacc as bacc
    import concourse.tile as tile
    import concourse.mybir as mybir

    f32 = mybir.dt.float32
    nc = bacc.Bacc("TRN2", target_bir_lowering=False, debug=False,
                   num_devices=N_CORES)
    xt_d = nc.dram_tensor("xt", [D_MODEL, T_CORE], f32,
                          kind="ExternalInput").ap()
    wr_d = nc.dram_tensor("wr", [D_MODEL, N_EXP], f32,
                          kind="ExternalInput").ap()
    gates_d = nc.dram_tensor("gates", [T_CORE, N_EXP], f32,
                             kind="ExternalOutput").ap()

    with tile.TileContext(nc) as tc:
        with ExitStack() as ctx:
            def body():
                sb = ctx.enter_context(tc.tile_pool(name="sb", bufs=1))
                evpool = ctx.enter_context(tc.tile_pool(name="ev", bufs=3))
                ps = ctx.enter_context(
                    tc.tile_pool(name="ps", bufs=3, space="PSUM"))

                wr_sb = sb.tile([P, N_DCH * N_EXP], f32, tag="wr")
                nc.sync.dma_start(
                    wr_sb[:].rearrange("p (dc e) -> p dc e", dc=N_DCH),
                    wr_d.rearrange("(dc p) e -> p dc e", p=P))
                # xt loaded in 4 token-quarter chunks across DMA engines so
                # router matmuls overlap the tail of the load
                QT = T_CORE // 4
                engs = [nc.sync, nc.gpsimd, nc.scalar]
                xt_q = []
                for q in range(4):
                    xq = sb.tile([P, N_DCH * QT], f32, tag=f"xt{q}",
                                 name=f"xt{q}")
                    engs[q % 3].dma_start(
                        xq[:].rearrange("p (dc t) -> p dc t", dc=N_DCH),
                        xt_d[:, q * QT:(q + 1) * QT]
                        .rearrange("(dc p) t -> p dc t", p=P))
                    xt_q.append(xq)

                gates_sb = sb.tile([P, N_TCH * N_EXP], f32, tag="gates")
                TCH_PER_Q = N_TCH // 4
                for tch in range(N_TCH):
                    xq = xt_q[tch // TCH_PER_Q]
                    tq = (tch % TCH_PER_Q) * P
                    lg_ps = ps.tile([P, N_EXP], f32, tag="lg", space="PSUM")
                    for dc in range(N_DCH):
                        nc.tensor.matmul(
                            lg_ps[:],
                            xq[:, dc * QT + tq:dc * QT + tq + P],
                            wr_sb[:, dc * N_EXP:(dc + 1) * N_EXP],
                            start=(dc == 0), stop=(dc == N_DCH - 1))
                    lg = evpool.tile([P, N_EXP], f32, tag="lg_sb")
                    mx = evpool.tile([P, 1], f32, tag="mx")
                    nc.vector.reduce_max(mx[:], lg_ps[:],
                                         axis=mybir.AxisListType.X)
                    nc.vector.tensor_scalar_sub(lg[:], lg_ps[:], mx[:])
                    pr = evpool.tile([P, N_EXP], f32, tag="pr")
                    nc.scalar.activation(
                        pr[:], lg[:], mybir.ActivationFunctionType.Exp)
                    sm = evpool.tile([P, 1], f32, tag="sm")
                    nc.vector.reduce_sum(sm[:], pr[:],
                                         axis=mybir.AxisListType.X)
                    rc = evpool.tile([P, 1], f32, tag="rc")
                    nc.vector.reciprocal(rc[:], sm[:])
                    nc.vector.tensor_scalar_mul(pr[:], pr[:], rc[:])
                    m8 = evpool.tile([P, 8], f32, tag="m8")
                    nc.vector.max(m8[:], pr[:])
                    nc.vector.memset(m8[:, TOPK:], -1.0)
                    rep = evpool.tile([P, N_EXP], f32, tag="rep")
                    nc.vector.match_replace(rep[:], m8[:], pr[:], 0.0)
                    g = gates_sb[:, tch * N_EXP:(tch + 1) * N_EXP]
                    nc.vector.tensor_sub(g, pr[:], rep[:])

                nc.sync.dma_start(
                    gates_d.rearrange("(tc p) e -> p tc e", p=P),
                    gates_sb[:].rearrange("p (tc e) -> p tc e", tc=N_TCH))

            if reps == 1:
                body()
            else:
                with tc.For_i(0, reps, 1):
                    body()
    nc.compile()
    return nc


def build_ffn_kernel(cap: int, reps: int = 1):
    """Expert-parallel FFN: one expert per core, cap tokens (mult of 512)."""
    import concourse.bacc as bacc
    import concourse.tile as tile
    import concourse.mybir as mybir

    assert cap % 512 == 0
    n_tt = cap // 512
    n_sch = cap // P       # slot chunks of 128

    f32 = mybir.dt.float32
    f32r = mybir.dt.float32r
    nc = bacc.Bacc("TRN2", target_bir_lowering=False, debug=False,
                   num_devices=N_CORES)
    xet_d = nc.dram_tensor("xet", [D_MODEL, cap], f32r,
                           kind="ExternalInput").ap()
    w1_d = nc.dram_tensor("w1", [D_MODEL, H_HID], f32r,
                          kind="ExternalInput").ap()
    b1_d = nc.dram_tensor("b1", [H_HID], f32, kind="ExternalInput").ap()
    w2_d = nc.dram_tensor("w2", [H_HID, D_MODEL], f32r,
                          kind="ExternalInput").ap()
    b2_d = nc.dram_tensor("b2", [D_MODEL], f32r, kind="ExternalInput").ap()
    gv_d = nc.dram_tensor("gv", [cap], f32, kind="ExternalInput").ap()
    y_d = nc.dram_tensor("y", [cap, D_MODEL], f32,
                         kind="ExternalOutput").ap()

    with tile.TileContext(nc) as tc:
        with ExitStack() as ctx:
            def body():
                sb = ctx.enter_context(tc.tile_pool(name="sb", bufs=1))
                hpool = ctx.enter_context(tc.tile_pool(name="hp", bufs=3))
                ypool_sb = ctx.enter_context(tc.tile_pool(name="ysb",
                                                          bufs=3))
                ps = ctx.enter_context(
                    tc.tile_pool(name="ps", bufs=3, space="PSUM"))
                yps = ctx.enter_context(
                    tc.tile_pool(name="yp", bufs=1, space="PSUM"))

                engs = [nc.sync, nc.gpsimd, nc.scalar]
                # chunked resident loads spread across the DMA engines:
                # w1 per d-chunk (first - FC1 needs it), xet per slot-tile,
                # w2 per 4-h-chunk group
                w1_c = []
                for dc in range(N_DCH):
                    wt = sb.tile([P, H_HID], f32r, tag=f"w1_{dc}",
                                 name=f"w1_{dc}")
                    engs[dc % 3].dma_start(
                        wt[:], w1_d[dc * P:(dc + 1) * P, :])
                    w1_c.append(wt)
                xet_c = []
                for tt in range(n_tt):
                    xt_ = sb.tile([P, N_DCH * 512], f32r, tag=f"xet{tt}",
                                  name=f"xet{tt}")
                    engs[tt % 3].dma_start(
                        xt_[:].rearrange("p (dc t) -> p dc t", dc=N_DCH),
                        xet_d[:, tt * 512:(tt + 1) * 512]
                        .rearrange("(dc p) t -> p dc t", p=P))
                    xet_c.append(xt_)
                w2_c = []
                for hg in range(4):
                    wt = sb.tile([P, 4 * D_MODEL], f32r, tag=f"w2_{hg}",
                                 name=f"w2_{hg}")
                    engs[hg % 3].dma_start(
                        wt[:].rearrange("p (hc d) -> p hc d", hc=4),
                        w2_d[hg * 4 * P:(hg + 1) * 4 * P, :]
                        .rearrange("(hc p) d -> p hc d", p=P))
                    w2_c.append(wt)
                b1_sb = sb.tile([P, N_HCH], f32, tag="b1")
                nc.sync.dma_start(
                    b1_sb[:], b1_d.rearrange("(hc p) -> p hc", p=P))
                b2_sb = sb.tile([1, D_MODEL], f32r, tag="b2")
                nc.sync.dma_start(b2_sb[:], b2_d[None, :])
                gv_sb = sb.tile([P, n_sch], f32, tag="gv")
                nc.sync.dma_start(
                    gv_sb[:], gv_d.rearrange("(sc p) -> p sc", p=P))
                ones_f = sb.tile([1, P], f32, tag="ones_f")
                nc.vector.memset(ones_f[:], 1.0)
                ones_sb = sb.tile([1, P], f32r, tag="ones")
                nc.vector.tensor_copy(ones_sb[:], ones_f[:])

                import concourse.mybir as mybir
                for tt in range(n_tt):
                    y_ps = [yps.tile([P, D_MODEL], f32, tag=f"y{i}",
                                     name=f"y{i}", space="PSUM")
                            for i in range(4)]
                    for hc in range(N_HCH):
                        h_ps = ps.tile([P, 512], f32, tag="h", space="PSUM")
                        for dc in range(N_DCH):
                            nc.tensor.matmul(
                                h_ps[:],
                                w1_c[dc][:, hc * P:(hc + 1) * P],
                                xet_c[tt][:, dc * 512:(dc + 1) * 512],
                                start=(dc == 0), stop=(dc == N_DCH - 1))
                        h_sb = hpool.tile([P, 512], f32r, tag="h_sb")
                        nc.vector.tensor_scalar(
                            h_sb[:], h_ps[:],
                            b1_sb[:, hc:hc + 1], 0.0,
                            op0=mybir.AluOpType.add,
                            op1=mybir.AluOpType.max)
                        w2t = w2_c[hc // 4]
                        w2s = w2t[:, (hc % 4) * D_MODEL:
                                  (hc % 4 + 1) * D_MODEL]
                        for ts in range(4):
                            nc.tensor.matmul(
                                y_ps[ts][:],
                                h_sb[:, ts * P:(ts + 1) * P],
                                w2s,
                                start=(hc == 0), stop=False)
                    for ts in range(4):
                        nc.tensor.matmul(
                            y_ps[ts][:], ones_sb[:, :P], b2_sb[:],
                            start=False, stop=True)
                    for ts in range(4):
                        sc = tt * 4 + ts
                        y_sb = ypool_sb.tile([P, D_MODEL], f32, tag="ysb")
                        nc.scalar.activation(
                            y_sb[:], y_ps[ts][:],
                            mybir.ActivationFunctionType.Copy,
                            scale=gv_sb[:, sc:sc + 1])
                        nc.sync.dma_start(
                            y_d[sc * P:(sc + 1) * P, :], y_sb[:])

            if reps == 1:
                body()
            else:
                with tc.For_i(0, reps, 1):
                    body()
    nc.compile()
    return nc


def _router_in_maps(input_batch, Wr):
    return [{
        "xt": np.ascontiguousarray(
            input_batch[c * T_CORE:(c + 1) * T_CORE].T),
        "wr": np.ascontiguousarray(Wr),
    } for c in range(N_CORES)]


def _dispatch(input_batch, gates):
    """Build per-expert gathered inputs. Returns (cap, idx_list, in_maps)."""
    idx_list = [np.nonzero(gates[:, e])[0] for e in range(N_EXP)]
    max_cnt = max(len(ix) for ix in idx_list)
    cap = max(512, ((max_cnt + 511) // 512) * 512)
    in_maps = []
    for e in range(N_EXP):
        ix = idx_list[e]
        xe_t = np.zeros((D_MODEL, cap), np.float32)
        xe_t[:, :len(ix)] = input_batch[ix].T
        gv = np.zeros(cap, np.float32)
        gv[:len(ix)] = gates[ix, e]
        in_maps.append({"xet": xe_t, "gv": gv})
    return cap, idx_list, in_maps


def kernel(input_batch, Wr, W1, b1, W2, b2):
    from concourse import bass_utils

    input_batch = np.ascontiguousarray(input_batch, dtype=np.float32)
    Wr = np.ascontiguousarray(Wr, dtype=np.float32)
    W1 = np.ascontiguousarray(W1, dtype=np.float32)
    b1 = np.ascontiguousarray(b1, dtype=np.float32)
    W2 = np.ascontiguousarray(W2, dtype=np.float32)
    b2 = np.ascontiguousarray(b2, dtype=np.float32)

    # ---- launch 1: router ----
    if "router" not in _CACHE:
        _CACHE["router"] = build_router_kernel()
    res = bass_utils.run_bass_kernel_spmd(
        _CACHE["router"], _router_in_maps(input_batch, Wr),
        core_ids=list(range(N_CORES)))
    gates = np.concatenate([res.results[c]["gates"]
                            for c in range(N_CORES)], axis=0)

    # ---- host dispatch ----
    cap, idx_list, in_maps = _dispatch(input_batch, gates)

    # ---- launch 2: expert FFN ----
    key = ("ffn", cap)
    if key not in _CACHE:
        _CACHE[key] = build_ffn_kernel(cap)
    for e in range(N_EXP):
        in_maps[e]["w1"] = W1[e]
        in_maps[e]["b1"] = b1[e]
        in_maps[e]["w2"] = W2[e]
        in_maps[e]["b2"] = b2[e]
    res = bass_utils.run_bass_kernel_spmd(
        _CACHE[key], in_maps, core_ids=list(range(N_CORES)))

    # ---- host combine ----
    out = np.zeros((N_TOK, D_MODEL), np.float32)
    for e in range(N_EXP):
        ix = idx_list[e]
        out[ix] += res.results[e]["y"][:len(ix)]
    return out, np.float32(0.0)
